# revision 5
# baseline (speedup 1.0000x reference)
"""MoE runtime-experts kernel for 8 Trainium2 NeuronCores.

Expert-parallel: core e holds expert e's weights. Host routes tokens by
expert id (argsort), pads each expert batch to a common capacity C, and
each core computes y = gelu(x @ W1 + b1) @ W2 + b2 for its batch as
dense matmuls in a transposed layout:

    L1: hT[hid, tok]  = W1[in, hid].T-contract  xT[in, tok]
    L2: yT[out, tok]  = W2[hid, out].T-contract hT[hid, tok]

Weights stay stationary on the PE (lhsT), tokens are the moving dim, so
activations flow through both layers without any on-device transpose.
Inputs/weights are cast to bf16 on host (PSUM accumulates fp32).
"""

import numpy as np
import ml_dtypes

import concourse.bass as bass
import concourse.mybir as mybir
import concourse.tile as tile
from concourse import bacc
from concourse.bass_utils import run_bass_kernel_spmd

P = 128
N_CORES = 8
BF16 = mybir.dt.bfloat16
F32 = mybir.dt.float32

_nc_cache = {}


def _token_tile_widths(C):
    """Split C (multiple of 128) into near-equal 128-multiple chunks <=512.
    Equal chunks keep every matmul's moving dim >=256 (for C>=512), so the
    per-matmul LDWEIGHTS (107ns) stays hidden under the MM stream."""
    nch = -(-C // 512)
    base = (C // nch) // P * P
    widths = [base] * nch
    rem = (C - base * nch) // P
    for i in range(rem):
        widths[i] += P
    assert sum(widths) == C
    return widths


def _build_kernel(C, IN, HID, OUT, skip_in_dma=False, psum_bufs=8, repeat=1,
                  tile_w=256, PIPE=1, GROUP=True):
    K1, M1 = IN // P, HID // P
    K2, M2 = HID // P, OUT // P
    if tile_w is not None:
        widths = [tile_w] * (C // tile_w)
        if C % tile_w:
            widths.append(C % tile_w)
    else:
        widths = _token_tile_widths(C)

    nc = bacc.Bacc("TRN2", target_bir_lowering=False, debug=False,
                   num_devices=N_CORES)
    xT = nc.dram_tensor("xT", [IN, C], BF16, kind="ExternalInput")
    w1 = nc.dram_tensor("w1", [IN, HID], BF16, kind="ExternalInput")
    w2 = nc.dram_tensor("w2", [HID, OUT], BF16, kind="ExternalInput")
    b1 = nc.dram_tensor("b1", [HID], F32, kind="ExternalInput")
    b2 = nc.dram_tensor("b2", [OUT], F32, kind="ExternalInput")
    yT = nc.dram_tensor("yT", [OUT, C], F32, kind="ExternalOutput")

    with tile.TileContext(nc) as tc:
        with (
            tc.tile_pool(name="weights", bufs=1) as wpool,
            tc.tile_pool(name="xbuf", bufs=PIPE + 1) as xpool,
            tc.tile_pool(name="hbuf", bufs=PIPE + 1) as hpool,
            tc.tile_pool(name="obuf", bufs=2) as opool,
            tc.tile_pool(name="psum", bufs=psum_bufs, space="PSUM") as pspool,
        ):
            w1_sb = wpool.tile([P, K1, HID], BF16)
            w2_sb = wpool.tile([P, K2, OUT], BF16)
            b1_sb = wpool.tile([P, M1], F32)
            b2_sb = wpool.tile([P, M2], F32)

            xTr = xT.ap().rearrange("(k p) c -> p k c", p=P)
            w1r = w1.ap().rearrange("(k p) m -> p k m", p=P)
            w2r = w2.ap().rearrange("(k p) m -> p k m", p=P)

            # First token tile's x up front so PE can start ASAP.
            MAXW_ = max(widths)
            x_tiles = {}
            if repeat == 1:
                x_tiles[0] = xpool.tile([P, K1, MAXW_], BF16, tag="x",
                                        name="x_sb")
            if not skip_in_dma:
                if repeat == 1:
                    nc.sync.dma_start(x_tiles[0][:, :, :widths[0]],
                                      xTr[:, :, 0:widths[0]])
                # Biases are tiny and the first gelu needs b1 early.
                nc.sync.dma_start(b1_sb[:],
                                  b1.ap().rearrange("(m p) -> p m", p=P))
                nc.sync.dma_start(b2_sb[:],
                                  b2.ap().rearrange("(m p) -> p m", p=P))
                # w1 chunked by m-window (all k per DMA): the first L1 chain
                # only needs window 0 instead of all of w1 (8MB), and
                # few big DMAs keep the per-DMA issue overhead
                # (~0.7us on the SP sequencer) off the critical path.
                # A small leading window (128 cols, 0.25MB) lets the PE
                # start ~3us earlier than a 512-col one.
                mw_edges = [0, 128, 512]
                while mw_edges[-1] < HID:
                    mw_edges.append(min(HID, mw_edges[-1] + 512))
                for lo, hi in zip(mw_edges[:-1], mw_edges[1:]):
                    nc.sync.dma_start(w1_sb[:, :, lo:hi], w1r[:, :, lo:hi])
                # w2 is only needed ~halfway in; 4-k chunks, k-major order.
                KC = 4
                for kc in range(K2 // KC):
                    nc.sync.dma_start(w2_sb[:, kc * KC:(kc + 1) * KC],
                                      w2r[:, kc * KC:(kc + 1) * KC])

            starts_ = [sum(widths[:i]) for i in range(len(widths))]
            MAXW = max(widths)

            def l1_phase(it):
                NW, n0 = widths[it], starts_[it]
                if it in x_tiles:
                    x_sb = x_tiles[it]
                else:
                    x_sb = xpool.tile([P, K1, MAXW], BF16, tag="x",
                                      name="x_sb")
                    if not skip_in_dma:
                        nc.sync.dma_start(x_sb[:, :, :NW],
                                          xTr[:, :, n0:n0 + NW])
                h_sb = hpool.tile([P, K2, MAXW], BF16, tag="h", name="h_sb")
                if GROUP and NW <= 128:
                    # Pack 4 accumulation chains into one PSUM bank so the
                    # slot-WAR sem wait is paid once per 4 chains.
                    for mg in range(0, M1, 4):
                        ps = pspool.tile([P, 512], F32, tag="ps", name="ps")
                        for mi in range(4):
                            m = mg + mi
                            for k in range(K1):
                                nc.tensor.matmul(
                                    ps[:, mi * P:mi * P + NW],
                                    w1_sb[:, k, bass.ts(m, P)],
                                    x_sb[:, k, :NW],
                                    start=(k == 0),
                                    stop=(k == K1 - 1),
                                )
                        for mi in range(4):
                            m = mg + mi
                            nc.scalar.activation(
                                h_sb[:, m, :NW],
                                ps[:, mi * P:mi * P + NW],
                                mybir.ActivationFunctionType.Gelu,
                                bias=b1_sb[:, m:m + 1],
                            )
                else:
                    for m in range(M1):
                        ps = pspool.tile([P, 512], F32, tag="ps", name="ps")
                        for k in range(K1):
                            nc.tensor.matmul(
                                ps[:, :NW],
                                w1_sb[:, k, bass.ts(m, P)],
                                x_sb[:, k, :NW],
                                start=(k == 0),
                                stop=(k == K1 - 1),
                            )
                        nc.scalar.activation(
                            h_sb[:, m, :NW],
                            ps[:, :NW],
                            mybir.ActivationFunctionType.Gelu,
                            bias=b1_sb[:, m:m + 1],
                        )
                return h_sb

            yTr = yT.ap().rearrange("(m p) c -> p m c", p=P)

            def l2_phase(it, h_sb):
                NW, n0 = widths[it], starts_[it]
                o_sb = opool.tile([P, M2, MAXW], F32, tag="o", name="o_sb")
                if GROUP and NW <= 128:
                    for mg in range(0, M2, 4):
                        ps = pspool.tile([P, 512], F32, tag="ps", name="ps")
                        for mi in range(4):
                            m = mg + mi
                            for k in range(K2):
                                nc.tensor.matmul(
                                    ps[:, mi * P:mi * P + NW],
                                    w2_sb[:, k, bass.ts(m, P)],
                                    h_sb[:, k, :NW],
                                    start=(k == 0),
                                    stop=(k == K2 - 1),
                                )
                        for mi in range(4):
                            m = mg + mi
                            nc.vector.tensor_tensor(
                                o_sb[:, m, :NW],
                                ps[:, mi * P:mi * P + NW],
                                b2_sb[:, m:m + 1].to_broadcast((P, NW)),
                                mybir.AluOpType.add,
                            )
                else:
                    for m in range(M2):
                        ps = pspool.tile([P, 512], F32, tag="ps", name="ps")
                        for k in range(K2):
                            nc.tensor.matmul(
                                ps[:, :NW],
                                w2_sb[:, k, bass.ts(m, P)],
                                h_sb[:, k, :NW],
                                start=(k == 0),
                                stop=(k == K2 - 1),
                            )
                        nc.vector.tensor_tensor(
                            o_sb[:, m, :NW],
                            ps[:, :NW],
                            b2_sb[:, m:m + 1].to_broadcast((P, NW)),
                            mybir.AluOpType.add,
                        )
                nc.sync.dma_start(yTr[:, :, n0:n0 + NW], o_sb[:, :, :NW])

            def body():
                # Software pipeline: L1 runs PIPE tiles ahead of L2 so the
                # w2 weight DMA tail hides behind L1 compute at startup.
                n_t = len(widths)
                depth = min(PIPE, n_t)
                hs = {}
                for it in range(depth):
                    hs[it] = l1_phase(it)
                for j in range(n_t):
                    if j + depth < n_t:
                        hs[j + depth] = l1_phase(j + depth)
                    l2_phase(j, hs.pop(j))

            if repeat == 1:
                body()
            else:
                with tc.For_i(0, repeat, 1, name="rep"):
                    body()
    nc.compile()
    return nc


def _get_kernel(C, IN, HID, OUT):
    key = (C, IN, HID, OUT)
    if key not in _nc_cache:
        _nc_cache[key] = _build_kernel(C, IN, HID, OUT)
    return _nc_cache[key]


def prepare_in_maps(inputs):
    """Host-side routing: sort tokens by expert, pad to capacity C,
    build per-core input maps. Returns (in_maps, meta)."""
    x = np.ascontiguousarray(np.asarray(inputs["x"], dtype=np.float32))
    idx = np.asarray(inputs["indices_s"]).astype(np.int64)
    w1 = np.asarray(inputs["weight1"], dtype=np.float32)
    w2 = np.asarray(inputs["weight2"], dtype=np.float32)
    b1 = np.asarray(inputs["bias1"], dtype=np.float32)
    b2 = np.asarray(inputs["bias2"], dtype=np.float32)

    T = x.shape[0]
    E, IN, HID = w1.shape
    OUT = w2.shape[2]
    assert E == N_CORES
    bf = ml_dtypes.bfloat16

    order = np.argsort(idx, kind="stable")
    counts = np.bincount(idx, minlength=E)
    starts = np.zeros(E + 1, dtype=np.int64)
    starts[1:] = np.cumsum(counts)
    # Tokens are the matmul moving dim — no 128 alignment needed. Pad the
    # per-expert capacity to 16 only (DMA-friendly), not 128: with counts
    # ~1024+eps this cuts ~11% of padded compute.
    C = max(-(-int(counts.max()) // 16) * 16, 16)

    xbf = x.astype(bf)
    in_maps = []
    for e in range(E):
        toks = order[starts[e]:starts[e + 1]]
        xTe = np.zeros((IN, C), dtype=bf)
        if len(toks):
            xTe[:, :len(toks)] = xbf[toks].T
        in_maps.append({
            "xT": xTe,
            "w1": np.ascontiguousarray(w1[e]).astype(bf),
            "w2": np.ascontiguousarray(w2[e]).astype(bf),
            "b1": np.ascontiguousarray(b1[e]),
            "b2": np.ascontiguousarray(b2[e]),
        })
    meta = {"key": (C, IN, HID, OUT), "order": order, "starts": starts,
            "T": T, "OUT": OUT}
    return in_maps, meta


def scatter_output(inputs, yT_all, meta):
    """Scatter per-core yT [E, OUT, C] back to [T, 1, OUT] fp32."""
    order, starts = meta["order"], meta["starts"]
    out = np.empty((meta["T"], meta["OUT"]), dtype=np.float32)
    for e in range(N_CORES):
        toks = order[starts[e]:starts[e + 1]]
        if len(toks):
            out[toks] = yT_all[e][:, :len(toks)].T
    return out[:, None, :]


def kernel(**inputs):
    in_maps, meta = prepare_in_maps(inputs)
    nc = _get_kernel(*meta["key"])
    res = run_bass_kernel_spmd(nc, in_maps, core_ids=list(range(N_CORES)),
                               trace=False)
    yT_all = np.stack([res.results[e]["yT"] for e in range(N_CORES)])
    return scatter_output(inputs, yT_all, meta)



# revision 15
# speedup vs baseline: 1.0061x; 1.0061x over previous
"""MoE runtime-experts kernel for 8 Trainium2 NeuronCores.

Expert-parallel: core e holds expert e's weights. Host routes tokens by
expert id (argsort), pads each expert batch to a common capacity C, and
each core computes y = gelu(x @ W1 + b1) @ W2 + b2 for its batch as
dense matmuls in a transposed layout:

    L1: hT[hid, tok]  = W1[in, hid].T-contract  xT[in, tok]
    L2: yT[out, tok]  = W2[hid, out].T-contract hT[hid, tok]

Weights stay stationary on the PE (lhsT), tokens are the moving dim, so
activations flow through both layers without any on-device transpose.
Inputs/weights are cast to bf16 on host (PSUM accumulates fp32).
"""

import numpy as np
import ml_dtypes

import concourse.bass as bass
import concourse.mybir as mybir
import concourse.tile as tile
from concourse import bacc
from concourse.bass_utils import run_bass_kernel_spmd

P = 128
N_CORES = 8
BF16 = mybir.dt.bfloat16
F32 = mybir.dt.float32

_nc_cache = {}


def _token_tile_widths(C, maxw=272):
    """Split C into near-equal chunks of <=maxw tokens. Tokens are the
    moving dim, so widths need no 128 alignment. Near-equal (vs 512+tail)
    keeps the per-MM NX overhead amortized evenly and avoids a skinny
    tail tile whose 512 matmuls would each pay the ~25ns issue floor."""
    nch = -(-C // maxw)
    base, rem = divmod(C, nch)
    widths = [base + (1 if i < rem else 0) for i in range(nch)]
    assert sum(widths) == C
    return widths


def _build_kernel(C, IN, HID, OUT, skip_in_dma=False, psum_bufs=8, repeat=1,
                  tile_w=None, PIPE=1, GROUP=True):
    K1, M1 = IN // P, HID // P
    K2, M2 = HID // P, OUT // P
    if tile_w is not None:
        widths = [tile_w] * (C // tile_w)
        if C % tile_w:
            widths.append(C % tile_w)
    else:
        widths = _token_tile_widths(C)

    nc = bacc.Bacc("TRN2", target_bir_lowering=False, debug=False,
                   num_devices=N_CORES)
    xT = nc.dram_tensor("xT", [IN, C], BF16, kind="ExternalInput")
    w1 = nc.dram_tensor("w1", [IN, HID], BF16, kind="ExternalInput")
    w2 = nc.dram_tensor("w2", [HID, OUT], BF16, kind="ExternalInput")
    b1 = nc.dram_tensor("b1", [HID], F32, kind="ExternalInput")
    b2 = nc.dram_tensor("b2", [OUT], F32, kind="ExternalInput")
    yT = nc.dram_tensor("yT", [OUT, C], F32, kind="ExternalOutput")

    with tile.TileContext(nc) as tc:
        with (
            tc.tile_pool(name="weights", bufs=1) as wpool,
            tc.tile_pool(name="hbuf", bufs=PIPE + 1) as hpool,
            tc.tile_pool(name="obuf", bufs=2) as opool,
            tc.tile_pool(name="psum", bufs=psum_bufs, space="PSUM") as pspool,
        ):
            w1_sb = wpool.tile([P, K1, HID], BF16)
            w2_sb = wpool.tile([P, K2, OUT], BF16)
            b1_sb = wpool.tile([P, M1], F32)
            b2_sb = wpool.tile([P, M2], F32)
            # All of x stays resident (2.1MB): its DMA is issued before the
            # bulk weight windows so no L1 tile ever waits behind the 16MB
            # weight stream on the shared HBM bandwidth.
            x_sb = wpool.tile([P, K1, C], BF16)

            xTr = xT.ap().rearrange("(k p) c -> p k c", p=P)
            w1r = w1.ap().rearrange("(k p) m -> p k m", p=P)
            w2r = w2.ap().rearrange("(k p) m -> p k m", p=P)

            def x_dma(head):
                # slice 0 alone first: the PE can start on tile 0 ~1.5us in
                nc.sync.dma_start(x_sb[:, :, :widths[0]],
                                  xTr[:, :, :widths[0]])
                if not head:
                    nc.sync.dma_start(x_sb[:, :, widths[0]:],
                                      xTr[:, :, widths[0]:])

            if not skip_in_dma:
                if repeat == 1:
                    x_dma(head=True)
                # w1 chunked by m-window (all k per DMA): the first L1 chain
                # only needs window 0 instead of all of w1 (8MB), and
                # few big DMAs keep the per-DMA issue overhead
                # (~0.7us on the SP sequencer) off the critical path.
                # A small leading window (128 cols, 0.25MB) lets the PE
                # start ~3us earlier than a 512-col one.
                mw_edges = [0, 128, 512]
                while mw_edges[-1] < HID:
                    mw_edges.append(min(HID, mw_edges[-1] + 512))
                nc.sync.dma_start(w1_sb[:, :, :128], w1r[:, :, :128])
                # Biases are tiny and the first gelu needs b1 early.
                nc.sync.dma_start(b1_sb[:],
                                  b1.ap().rearrange("(m p) -> p m", p=P))
                nc.sync.dma_start(b2_sb[:],
                                  b2.ap().rearrange("(m p) -> p m", p=P))
                for lo, hi in zip(mw_edges[1:-1], mw_edges[2:]):
                    nc.sync.dma_start(w1_sb[:, :, lo:hi], w1r[:, :, lo:hi])
                # x remainder after w1: w1 windows feed L1 tile 0 chain-by-
                # chain (demand 0.9us/window vs supply 0.7us), while the x
                # remainder isn't read until L1 tile 1 (~34us in).
                if repeat == 1:
                    nc.sync.dma_start(x_sb[:, :, widths[0]:],
                                      xTr[:, :, widths[0]:])
                # w2 m-major windows: L2 chains for out-block mg only need
                # the window containing mg, so the first L2 chain can start
                # as soon as its window lands instead of waiting for all
                # 8MB of w2.
                MW = 512
                for mw in range(OUT // MW):
                    nc.sync.dma_start(
                        w2_sb[:, :, mw * MW:(mw + 1) * MW],
                        w2r[:, :, mw * MW:(mw + 1) * MW])

            starts_ = [sum(widths[:i]) for i in range(len(widths))]
            MAXW = max(widths)

            def l1_phase(it):
                NW, n0 = widths[it], starts_[it]
                h_sb = hpool.tile([P, K2, MAXW], BF16, tag="h", name="h_sb")
                # Pack as many accumulation chains as fit into one PSUM
                # bank (512 fp32) so the slot-WAR sem wait is amortized.
                spb = 4 if NW <= 128 else (2 if NW <= 256 else 1)
                slot = 512 // spb
                for mg in range(0, M1, spb):
                    ps = pspool.tile([P, 512], F32, tag="ps", name="ps")
                    for mi in range(spb):
                        m = mg + mi
                        for k in range(K1):
                            nc.tensor.matmul(
                                ps[:, mi * slot:mi * slot + NW],
                                w1_sb[:, k, bass.ts(m, P)],
                                x_sb[:, k, n0:n0 + NW],
                                start=(k == 0),
                                stop=(k == K1 - 1),
                            )
                    for mi in range(spb):
                        m = mg + mi
                        nc.scalar.activation(
                            h_sb[:, m, :NW],
                            ps[:, mi * slot:mi * slot + NW],
                            mybir.ActivationFunctionType.Gelu,
                            bias=b1_sb[:, m:m + 1],
                        )
                return h_sb

            yTr = yT.ap().rearrange("(m p) c -> p m c", p=P)

            def l2_phase(it, h_sb):
                NW, n0 = widths[it], starts_[it]
                o_sb = opool.tile([P, M2, MAXW], F32, tag="o", name="o_sb")
                spb = 4 if NW <= 128 else (2 if NW <= 256 else 1)
                slot = 512 // spb
                for mg in range(0, M2, spb):
                    ps = pspool.tile([P, 512], F32, tag="ps", name="ps")
                    for mi in range(spb):
                        m = mg + mi
                        for k in range(K2):
                            nc.tensor.matmul(
                                ps[:, mi * slot:mi * slot + NW],
                                w2_sb[:, k, bass.ts(m, P)],
                                h_sb[:, k, :NW],
                                start=(k == 0),
                                stop=(k == K2 - 1),
                            )
                    for mi in range(spb):
                        m = mg + mi
                        nc.vector.tensor_tensor(
                            o_sb[:, m, :NW],
                            ps[:, mi * slot:mi * slot + NW],
                            b2_sb[:, m:m + 1].to_broadcast((P, NW)),
                            mybir.AluOpType.add,
                        )
                    # Per-group output DMA overlaps the remaining chains'
                    # compute — shrinks the end-of-kernel drain.
                    nc.sync.dma_start(yTr[:, mg:mg + spb, n0:n0 + NW],
                                      o_sb[:, mg:mg + spb, :NW])

            def body():
                # Software pipeline: L1 runs PIPE tiles ahead of L2 so the
                # w2 weight DMA tail hides behind L1 compute at startup.
                if repeat > 1 and not skip_in_dma:
                    x_dma(head=False)
                n_t = len(widths)
                depth = min(PIPE, n_t)
                hs = {}
                for it in range(depth):
                    hs[it] = l1_phase(it)
                for j in range(n_t):
                    if j + depth < n_t:
                        hs[j + depth] = l1_phase(j + depth)
                    l2_phase(j, hs.pop(j))

            if repeat == 1:
                body()
            else:
                with tc.For_i(0, repeat, 1, name="rep"):
                    body()
    nc.compile()
    return nc


def _get_kernel(C, IN, HID, OUT):
    key = (C, IN, HID, OUT)
    if key not in _nc_cache:
        _nc_cache[key] = _build_kernel(C, IN, HID, OUT)
    return _nc_cache[key]


def prepare_in_maps(inputs):
    """Host-side routing: sort tokens by expert, pad to capacity C,
    build per-core input maps. Returns (in_maps, meta)."""
    x = np.ascontiguousarray(np.asarray(inputs["x"], dtype=np.float32))
    idx = np.asarray(inputs["indices_s"]).astype(np.int64)
    w1 = np.asarray(inputs["weight1"], dtype=np.float32)
    w2 = np.asarray(inputs["weight2"], dtype=np.float32)
    b1 = np.asarray(inputs["bias1"], dtype=np.float32)
    b2 = np.asarray(inputs["bias2"], dtype=np.float32)

    T = x.shape[0]
    E, IN, HID = w1.shape
    OUT = w2.shape[2]
    assert E == N_CORES
    bf = ml_dtypes.bfloat16

    order = np.argsort(idx, kind="stable")
    counts = np.bincount(idx, minlength=E)
    starts = np.zeros(E + 1, dtype=np.int64)
    starts[1:] = np.cumsum(counts)
    # Tokens are the matmul moving dim — no 128 alignment needed. Pad the
    # per-expert capacity to 16 only (DMA-friendly), not 128: with counts
    # ~1024+eps this cuts ~11% of padded compute.
    C = max(-(-int(counts.max()) // 16) * 16, 16)

    xbf = x.astype(bf)
    in_maps = []
    for e in range(E):
        toks = order[starts[e]:starts[e + 1]]
        xTe = np.zeros((IN, C), dtype=bf)
        if len(toks):
            xTe[:, :len(toks)] = xbf[toks].T
        in_maps.append({
            "xT": xTe,
            "w1": np.ascontiguousarray(w1[e]).astype(bf),
            "w2": np.ascontiguousarray(w2[e]).astype(bf),
            "b1": np.ascontiguousarray(b1[e]),
            "b2": np.ascontiguousarray(b2[e]),
        })
    meta = {"key": (C, IN, HID, OUT), "order": order, "starts": starts,
            "T": T, "OUT": OUT}
    return in_maps, meta


def scatter_output(inputs, yT_all, meta):
    """Scatter per-core yT [E, OUT, C] back to [T, 1, OUT] fp32."""
    order, starts = meta["order"], meta["starts"]
    out = np.empty((meta["T"], meta["OUT"]), dtype=np.float32)
    for e in range(N_CORES):
        toks = order[starts[e]:starts[e + 1]]
        if len(toks):
            out[toks] = yT_all[e][:, :len(toks)].T
    return out[:, None, :]


def kernel(**inputs):
    in_maps, meta = prepare_in_maps(inputs)
    nc = _get_kernel(*meta["key"])
    res = run_bass_kernel_spmd(nc, in_maps, core_ids=list(range(N_CORES)),
                               trace=False)
    yT_all = np.stack([res.results[e]["yT"] for e in range(N_CORES)])
    return scatter_output(inputs, yT_all, meta)



# revision 20
# speedup vs baseline: 1.0448x; 1.0385x over previous
"""MoE runtime-experts kernel for 8 Trainium2 NeuronCores.

Expert-parallel: core e holds expert e's weights. Host routes tokens by
expert id (argsort), pads each expert batch to a common capacity C, and
each core computes y = gelu(x @ W1 + b1) @ W2 + b2 for its batch as
dense matmuls in a transposed layout:

    L1: hT[hid, tok]  = W1[in, hid].T-contract  xT[in, tok]
    L2: yT[out, tok]  = W2[hid, out].T-contract hT[hid, tok]

Weights stay stationary on the PE (lhsT), tokens are the moving dim, so
activations flow through both layers without any on-device transpose.
Inputs/weights are cast to bf16 on host (PSUM accumulates fp32).
"""

import numpy as np
import ml_dtypes

import concourse.bass as bass
import concourse.mybir as mybir
import concourse.tile as tile
from concourse import bacc
from concourse.bass_utils import run_bass_kernel_spmd

P = 128
N_CORES = 8
BF16 = mybir.dt.bfloat16
F32 = mybir.dt.float32

_nc_cache = {}


def _token_tile_widths(C, maxw=272):
    """Split C into near-equal chunks of <=maxw tokens. Tokens are the
    moving dim, so widths need no 128 alignment. Near-equal (vs 512+tail)
    keeps the per-MM NX overhead amortized evenly and avoids a skinny
    tail tile whose 512 matmuls would each pay the ~25ns issue floor."""
    nch = -(-C // maxw)
    base, rem = divmod(C, nch)
    widths = [base + (1 if i < rem else 0) for i in range(nch)]
    assert sum(widths) == C
    return widths


def _build_kernel(C, IN, HID, OUT, skip_in_dma=False, psum_bufs=8, repeat=1,
                  tile_w=None, PIPE=1, GROUP=True, skip_out_dma=False):
    K1, M1 = IN // P, HID // P
    K2, M2 = HID // P, OUT // P
    if tile_w is not None:
        widths = [tile_w] * (C // tile_w)
        if C % tile_w:
            widths.append(C % tile_w)
    else:
        widths = _token_tile_widths(C)

    nc = bacc.Bacc("TRN2", target_bir_lowering=False, debug=False,
                   num_devices=N_CORES)
    xT = nc.dram_tensor("xT", [IN, C], BF16, kind="ExternalInput")
    w1 = nc.dram_tensor("w1", [IN, HID], BF16, kind="ExternalInput")
    w2 = nc.dram_tensor("w2", [HID, OUT], BF16, kind="ExternalInput")
    b1 = nc.dram_tensor("b1", [HID], F32, kind="ExternalInput")
    b2 = nc.dram_tensor("b2", [OUT], F32, kind="ExternalInput")
    yT = nc.dram_tensor("yT", [OUT, C], F32, kind="ExternalOutput")

    with tile.TileContext(nc) as tc:
        with (
            tc.tile_pool(name="weights", bufs=1) as wpool,
            tc.tile_pool(name="hbuf", bufs=PIPE + 1) as hpool,
            tc.tile_pool(name="obuf", bufs=2) as opool,
            tc.tile_pool(name="psum", bufs=psum_bufs, space="PSUM") as pspool,
        ):
            w1_sb = wpool.tile([P, K1, HID], BF16)
            w2_sb = wpool.tile([P, K2, OUT], BF16)
            b1_sb = wpool.tile([P, M1], F32)
            b2_sb = wpool.tile([P, M2], F32)
            # All of x stays resident (2.1MB): its DMA is issued before the
            # bulk weight windows so no L1 tile ever waits behind the 16MB
            # weight stream on the shared HBM bandwidth.
            x_sb = wpool.tile([P, K1, C], BF16)

            xTr = xT.ap().rearrange("(k p) c -> p k c", p=P)
            w1r = w1.ap().rearrange("(k p) m -> p k m", p=P)
            w2r = w2.ap().rearrange("(k p) m -> p k m", p=P)

            def x_dma(head):
                # slice 0 first, on the Activation HWDGE queue — runs in
                # parallel with w1 window 0 on the SP queue.
                W0 = widths[0]
                nc.scalar.dma_start(x_sb[:, :, :W0], xTr[:, :, :W0])
                if not head:
                    nc.sync.dma_start(x_sb[:, :, widths[0]:],
                                      xTr[:, :, widths[0]:])

            if not skip_in_dma:
                if repeat == 1:
                    x_dma(head=True)
                # w1 chunked by m-window (all k per DMA): the first L1 chain
                # only needs window 0 instead of all of w1 (8MB), and
                # few big DMAs keep the per-DMA issue overhead
                # (~0.7us on the SP sequencer) off the critical path.
                # Small leading windows let the PE start ~3us earlier than
                # 512-col ones would.
                mw_edges = [0, 128, 256, 512]
                while mw_edges[-1] < HID:
                    mw_edges.append(min(HID, mw_edges[-1] + 512))
                nc.sync.dma_start(w1_sb[:, :, :128], w1r[:, :, :128])
                # Biases are tiny and the first gelu needs b1 early; they
                # ride the Activation queue alongside x.
                nc.scalar.dma_start(b1_sb[:],
                                    b1.ap().rearrange("(m p) -> p m", p=P))
                nc.scalar.dma_start(b2_sb[:],
                                    b2.ap().rearrange("(m p) -> p m", p=P))
                for lo, hi in zip(mw_edges[1:-1], mw_edges[2:]):
                    nc.sync.dma_start(w1_sb[:, :, lo:hi], w1r[:, :, lo:hi])
                # x remainder after w1: w1 windows feed L1 tile 0 chain-by-
                # chain (demand 0.9us/window vs supply 0.7us), while the x
                # remainder isn't read until L1 tile 1 (~34us in).
                if repeat == 1:
                    nc.sync.dma_start(x_sb[:, :, widths[0]:],
                                      xTr[:, :, widths[0]:])
                # w2 m-major windows: L2 chains for out-block mg only need
                # the window containing mg, so the first L2 chain can start
                # as soon as its window lands instead of waiting for all
                # 8MB of w2.
                MW = 512
                for mw in range(OUT // MW):
                    nc.sync.dma_start(
                        w2_sb[:, :, mw * MW:(mw + 1) * MW],
                        w2r[:, :, mw * MW:(mw + 1) * MW])

            starts_ = [sum(widths[:i]) for i in range(len(widths))]
            MAXW = max(widths)

            def l1_phase(it):
                NW, n0 = widths[it], starts_[it]
                h_sb = hpool.tile([P, K2, MAXW], BF16, tag="h", name="h_sb")
                # Pack as many accumulation chains as fit into one PSUM
                # bank (512 fp32) so the slot-WAR sem wait is amortized.
                spb = 4 if NW <= 128 else (2 if NW <= 256 else 1)
                slot = 512 // spb
                for mg in range(0, M1, spb):
                    ps = pspool.tile([P, 512], F32, tag="ps", name="ps")
                    for mi in range(spb):
                        m = mg + mi
                        for k in range(K1):
                            nc.tensor.matmul(
                                ps[:, mi * slot:mi * slot + NW],
                                w1_sb[:, k, bass.ts(m, P)],
                                x_sb[:, k, n0:n0 + NW],
                                start=(k == 0),
                                stop=(k == K1 - 1),
                            )
                    for mi in range(spb):
                        m = mg + mi
                        nc.scalar.activation(
                            h_sb[:, m, :NW],
                            ps[:, mi * slot:mi * slot + NW],
                            mybir.ActivationFunctionType.Gelu,
                            bias=b1_sb[:, m:m + 1],
                        )
                return h_sb

            yTr = yT.ap().rearrange("(m p) c -> p m c", p=P)

            def l2_phase(it, h_sb):
                NW, n0 = widths[it], starts_[it]
                o_sb = opool.tile([P, M2, MAXW], F32, tag="o", name="o_sb")
                spb = 4 if NW <= 128 else (2 if NW <= 256 else 1)
                slot = 512 // spb
                for mg in range(0, M2, spb):
                    ps = pspool.tile([P, 512], F32, tag="ps", name="ps")
                    for mi in range(spb):
                        m = mg + mi
                        for k in range(K2):
                            nc.tensor.matmul(
                                ps[:, mi * slot:mi * slot + NW],
                                w2_sb[:, k, bass.ts(m, P)],
                                h_sb[:, k, :NW],
                                start=(k == 0),
                                stop=(k == K2 - 1),
                            )
                    for mi in range(spb):
                        m = mg + mi
                        nc.vector.tensor_tensor(
                            o_sb[:, m, :NW],
                            ps[:, mi * slot:mi * slot + NW],
                            b2_sb[:, m:m + 1].to_broadcast((P, NW)),
                            mybir.AluOpType.add,
                        )
                    # Per-group output DMA overlaps the remaining chains'
                    # compute — shrinks the end-of-kernel drain.
                    if not skip_out_dma:
                        nc.sync.dma_start(yTr[:, mg:mg + spb, n0:n0 + NW],
                                          o_sb[:, mg:mg + spb, :NW])

            def body():
                # Software pipeline: L1 runs PIPE tiles ahead of L2 so the
                # w2 weight DMA tail hides behind L1 compute at startup.
                if repeat > 1 and not skip_in_dma:
                    x_dma(head=False)
                n_t = len(widths)
                depth = min(PIPE, n_t)
                hs = {}
                for it in range(depth):
                    hs[it] = l1_phase(it)
                for j in range(n_t):
                    if j + depth < n_t:
                        hs[j + depth] = l1_phase(j + depth)
                    l2_phase(j, hs.pop(j))

            if repeat == 1:
                body()
            else:
                with tc.For_i(0, repeat, 1, name="rep"):
                    body()
    nc.compile()
    return nc


def _get_kernel(C, IN, HID, OUT):
    key = (C, IN, HID, OUT)
    if key not in _nc_cache:
        _nc_cache[key] = _build_kernel(C, IN, HID, OUT)
    return _nc_cache[key]


def prepare_in_maps(inputs):
    """Host-side routing: sort tokens by expert, pad to capacity C,
    build per-core input maps. Returns (in_maps, meta)."""
    x = np.ascontiguousarray(np.asarray(inputs["x"], dtype=np.float32))
    idx = np.asarray(inputs["indices_s"]).astype(np.int64)
    w1 = np.asarray(inputs["weight1"], dtype=np.float32)
    w2 = np.asarray(inputs["weight2"], dtype=np.float32)
    b1 = np.asarray(inputs["bias1"], dtype=np.float32)
    b2 = np.asarray(inputs["bias2"], dtype=np.float32)

    T = x.shape[0]
    E, IN, HID = w1.shape
    OUT = w2.shape[2]
    assert E == N_CORES
    bf = ml_dtypes.bfloat16

    order = np.argsort(idx, kind="stable")
    counts = np.bincount(idx, minlength=E)
    starts = np.zeros(E + 1, dtype=np.int64)
    starts[1:] = np.cumsum(counts)
    # Tokens are the matmul moving dim — no 128 alignment needed. Pad the
    # per-expert capacity to 16 only (DMA-friendly), not 128: with counts
    # ~1024+eps this cuts ~11% of padded compute.
    C = max(-(-int(counts.max()) // 16) * 16, 16)

    xbf = x.astype(bf)
    in_maps = []
    for e in range(E):
        toks = order[starts[e]:starts[e + 1]]
        xTe = np.zeros((IN, C), dtype=bf)
        if len(toks):
            xTe[:, :len(toks)] = xbf[toks].T
        in_maps.append({
            "xT": xTe,
            "w1": np.ascontiguousarray(w1[e]).astype(bf),
            "w2": np.ascontiguousarray(w2[e]).astype(bf),
            "b1": np.ascontiguousarray(b1[e]),
            "b2": np.ascontiguousarray(b2[e]),
        })
    meta = {"key": (C, IN, HID, OUT), "order": order, "starts": starts,
            "T": T, "OUT": OUT}
    return in_maps, meta


def scatter_output(inputs, yT_all, meta):
    """Scatter per-core yT [E, OUT, C] back to [T, 1, OUT] fp32."""
    order, starts = meta["order"], meta["starts"]
    out = np.empty((meta["T"], meta["OUT"]), dtype=np.float32)
    for e in range(N_CORES):
        toks = order[starts[e]:starts[e + 1]]
        if len(toks):
            out[toks] = yT_all[e][:, :len(toks)].T
    return out[:, None, :]


def kernel(**inputs):
    in_maps, meta = prepare_in_maps(inputs)
    nc = _get_kernel(*meta["key"])
    res = run_bass_kernel_spmd(nc, in_maps, core_ids=list(range(N_CORES)),
                               trace=False)
    yT_all = np.stack([res.results[e]["yT"] for e in range(N_CORES)])
    return scatter_output(inputs, yT_all, meta)



# revision 22
# speedup vs baseline: 1.2589x; 1.2049x over previous
"""MoE runtime-experts kernel for 8 Trainium2 NeuronCores.

Expert-parallel: core e holds expert e's weights. Host routes tokens by
expert id (argsort), pads each expert batch to a common capacity C, and
each core computes y = gelu(x @ W1 + b1) @ W2 + b2 for its batch as
dense matmuls in a transposed layout:

    L1: hT[hid, tok]  = W1[in, hid].T-contract  xT[in, tok]
    L2: yT[out, tok]  = W2[hid, out].T-contract hT[hid, tok]

Weights stay stationary on the PE (lhsT), tokens are the moving dim, so
activations flow through both layers without any on-device transpose.
Inputs/weights are cast to bf16 on host (PSUM accumulates fp32).

Key layout/scheduling choices (each measured against the cost-model
timeline and HW repeat-loop deltas):
  - C pads to 16, not 128: tokens are the moving dim and need no
    alignment; 128-padding burned 12.5% extra compute (max count 1040).
  - C splits into near-equal ~260-wide token tiles: maximizes the
    per-matmul moving width under the 512-fp32 PSUM bank limit and
    avoids a skinny tail tile whose 512 MMs pay the ~25ns issue floor.
    2048 matmuls total vs 4608 at 128-wide.
  - All of x resident in SBUF, DMA'd before the bulk weight windows:
    an x tile queued behind 16MB of weights stalled the PE ~18us.
  - w1 and w2 stream in m-major windows sized so arrival outpaces the
    PE's chain-by-chain demand; L1 runs one tile ahead of L2 to cover
    the w2 window.
  - A dozen dummy matmuls on zeroed scratch warm the PE's HAM clock
    gate (cold = 1.2GHz for ~3.4us) during the initial DMA wait.
"""

import numpy as np
import ml_dtypes

import concourse.bass as bass
import concourse.mybir as mybir
import concourse.tile as tile
from concourse import bacc
from concourse.bass_utils import run_bass_kernel_spmd

P = 128
N_CORES = 8
BF16 = mybir.dt.bfloat16
F32 = mybir.dt.float32

_nc_cache = {}


def _token_tile_widths(C, maxw=272):
    """Split C into near-equal chunks of <=maxw tokens. Tokens are the
    moving dim, so widths need no 128 alignment. Near-equal (vs 512+tail)
    keeps the per-MM NX overhead amortized evenly and avoids a skinny
    tail tile whose 512 matmuls would each pay the ~25ns issue floor."""
    nch = -(-C // maxw)
    base, rem = divmod(C, nch)
    widths = [base + (1 if i < rem else 0) for i in range(nch)]
    assert sum(widths) == C
    return widths


def _build_kernel(C, IN, HID, OUT, skip_in_dma=False, psum_bufs=8, repeat=1,
                  tile_w=None, PIPE=1, GROUP=True, skip_out_dma=False):
    K1, M1 = IN // P, HID // P
    K2, M2 = HID // P, OUT // P
    if tile_w is not None:
        widths = [tile_w] * (C // tile_w)
        if C % tile_w:
            widths.append(C % tile_w)
    else:
        widths = _token_tile_widths(C)

    nc = bacc.Bacc("TRN2", target_bir_lowering=False, debug=False,
                   num_devices=N_CORES)
    xT = nc.dram_tensor("xT", [IN, C], BF16, kind="ExternalInput")
    w1 = nc.dram_tensor("w1", [IN, HID], BF16, kind="ExternalInput")
    w2 = nc.dram_tensor("w2", [HID, OUT], BF16, kind="ExternalInput")
    b1 = nc.dram_tensor("b1", [HID], F32, kind="ExternalInput")
    b2 = nc.dram_tensor("b2", [OUT], F32, kind="ExternalInput")
    yT = nc.dram_tensor("yT", [OUT, C], F32, kind="ExternalOutput")

    with tile.TileContext(nc) as tc:
        with (
            tc.tile_pool(name="weights", bufs=1) as wpool,
            tc.tile_pool(name="hbuf", bufs=PIPE + 1) as hpool,
            tc.tile_pool(name="obuf", bufs=2) as opool,
            tc.tile_pool(name="psum", bufs=psum_bufs, space="PSUM") as pspool,
        ):
            w1_sb = wpool.tile([P, K1, HID], BF16)
            w2_sb = wpool.tile([P, K2, OUT], BF16)
            b1_sb = wpool.tile([P, M1], F32)
            b2_sb = wpool.tile([P, M2], F32)
            # All of x stays resident (2.1MB): its DMA is issued before the
            # bulk weight windows so no L1 tile ever waits behind the 16MB
            # weight stream on the shared HBM bandwidth.
            x_sb = wpool.tile([P, K1, C], BF16)

            xTr = xT.ap().rearrange("(k p) c -> p k c", p=P)
            w1r = w1.ap().rearrange("(k p) m -> p k m", p=P)
            w2r = w2.ap().rearrange("(k p) m -> p k m", p=P)

            if repeat == 1:
                # HAM warm-up: the PE clock-gate sits at 4/8 (1.2GHz) until
                # ~3.4us of sustained activity. Burn that window on dummy
                # matmuls over a zeroed scratch tile while the first x/w1
                # DMAs are still in flight, so real work starts at 2.4GHz.
                warm_sb = wpool.tile([P, 260], BF16)
                nc.vector.memset(warm_sb[:], 0.0)
                wps = pspool.tile([P, 512], F32, tag="ps", name="ps")
                for _ in range(12):
                    nc.tensor.matmul(wps[:, :260], warm_sb[:, :P],
                                     warm_sb[:], start=True, stop=True,
                                     skip_group_check=True)

            def x_dma(head):
                # slice 0 first, on the Activation HWDGE queue — runs in
                # parallel with w1 window 0 on the SP queue.
                W0 = widths[0]
                nc.scalar.dma_start(x_sb[:, :, :W0], xTr[:, :, :W0])
                if not head:
                    nc.sync.dma_start(x_sb[:, :, widths[0]:],
                                      xTr[:, :, widths[0]:])

            if not skip_in_dma:
                if repeat == 1:
                    x_dma(head=True)
                # w1 chunked by m-window (all k per DMA): the first L1 chain
                # only needs window 0 instead of all of w1 (8MB), and
                # few big DMAs keep the per-DMA issue overhead
                # (~0.7us on the SP sequencer) off the critical path.
                # Small leading windows let the PE start ~3us earlier than
                # 512-col ones would.
                mw_edges = [0, 128, 256, 512]
                while mw_edges[-1] < HID:
                    mw_edges.append(min(HID, mw_edges[-1] + 512))
                nc.sync.dma_start(w1_sb[:, :, :128], w1r[:, :, :128])
                # Biases are tiny and the first gelu needs b1 early; they
                # ride the Activation queue alongside x.
                nc.scalar.dma_start(b1_sb[:],
                                    b1.ap().rearrange("(m p) -> p m", p=P))
                nc.scalar.dma_start(b2_sb[:],
                                    b2.ap().rearrange("(m p) -> p m", p=P))
                for lo, hi in zip(mw_edges[1:-1], mw_edges[2:]):
                    nc.sync.dma_start(w1_sb[:, :, lo:hi], w1r[:, :, lo:hi])
                # x remainder after w1: w1 windows feed L1 tile 0 chain-by-
                # chain (demand 0.9us/window vs supply 0.7us), while the x
                # remainder isn't read until L1 tile 1 (~34us in).
                if repeat == 1:
                    nc.sync.dma_start(x_sb[:, :, widths[0]:],
                                      xTr[:, :, widths[0]:])
                # w2 m-major windows: L2 chains for out-block mg only need
                # the window containing mg, so the first L2 chain can start
                # as soon as its window lands instead of waiting for all
                # 8MB of w2.
                MW = 512
                for mw in range(OUT // MW):
                    nc.sync.dma_start(
                        w2_sb[:, :, mw * MW:(mw + 1) * MW],
                        w2r[:, :, mw * MW:(mw + 1) * MW])

            starts_ = [sum(widths[:i]) for i in range(len(widths))]
            MAXW = max(widths)

            def l1_phase(it):
                NW, n0 = widths[it], starts_[it]
                h_sb = hpool.tile([P, K2, MAXW], BF16, tag="h", name="h_sb")
                # Pack as many accumulation chains as fit into one PSUM
                # bank (512 fp32) so the slot-WAR sem wait is amortized.
                spb = 4 if NW <= 128 else (2 if NW <= 256 else 1)
                slot = 512 // spb
                for mg in range(0, M1, spb):
                    ps = pspool.tile([P, 512], F32, tag="ps", name="ps")
                    for mi in range(spb):
                        m = mg + mi
                        for k in range(K1):
                            nc.tensor.matmul(
                                ps[:, mi * slot:mi * slot + NW],
                                w1_sb[:, k, bass.ts(m, P)],
                                x_sb[:, k, n0:n0 + NW],
                                start=(k == 0),
                                stop=(k == K1 - 1),
                            )
                    for mi in range(spb):
                        m = mg + mi
                        nc.scalar.activation(
                            h_sb[:, m, :NW],
                            ps[:, mi * slot:mi * slot + NW],
                            mybir.ActivationFunctionType.Gelu,
                            bias=b1_sb[:, m:m + 1],
                        )
                return h_sb

            yTr = yT.ap().rearrange("(m p) c -> p m c", p=P)

            def l2_phase(it, h_sb):
                NW, n0 = widths[it], starts_[it]
                o_sb = opool.tile([P, M2, MAXW], F32, tag="o", name="o_sb")
                spb = 4 if NW <= 128 else (2 if NW <= 256 else 1)
                slot = 512 // spb
                for mg in range(0, M2, spb):
                    ps = pspool.tile([P, 512], F32, tag="ps", name="ps")
                    for mi in range(spb):
                        m = mg + mi
                        for k in range(K2):
                            nc.tensor.matmul(
                                ps[:, mi * slot:mi * slot + NW],
                                w2_sb[:, k, bass.ts(m, P)],
                                h_sb[:, k, :NW],
                                start=(k == 0),
                                stop=(k == K2 - 1),
                            )
                    for mi in range(spb):
                        m = mg + mi
                        nc.vector.tensor_tensor(
                            o_sb[:, m, :NW],
                            ps[:, mi * slot:mi * slot + NW],
                            b2_sb[:, m:m + 1].to_broadcast((P, NW)),
                            mybir.AluOpType.add,
                        )
                    # Per-group output DMA overlaps the remaining chains'
                    # compute — shrinks the end-of-kernel drain.
                    if not skip_out_dma:
                        nc.sync.dma_start(yTr[:, mg:mg + spb, n0:n0 + NW],
                                          o_sb[:, mg:mg + spb, :NW])

            def body():
                # Software pipeline: L1 runs PIPE tiles ahead of L2 so the
                # w2 weight DMA tail hides behind L1 compute at startup.
                if repeat > 1 and not skip_in_dma:
                    x_dma(head=False)
                n_t = len(widths)
                depth = min(PIPE, n_t)
                hs = {}
                for it in range(depth):
                    hs[it] = l1_phase(it)
                for j in range(n_t):
                    if j + depth < n_t:
                        hs[j + depth] = l1_phase(j + depth)
                    l2_phase(j, hs.pop(j))

            if repeat == 1:
                body()
            else:
                with tc.For_i(0, repeat, 1, name="rep"):
                    body()
    nc.compile()
    return nc


def _get_kernel(C, IN, HID, OUT):
    key = (C, IN, HID, OUT)
    if key not in _nc_cache:
        _nc_cache[key] = _build_kernel(C, IN, HID, OUT)
    return _nc_cache[key]


def prepare_in_maps(inputs):
    """Host-side routing: sort tokens by expert, pad to capacity C,
    build per-core input maps. Returns (in_maps, meta)."""
    x = np.ascontiguousarray(np.asarray(inputs["x"], dtype=np.float32))
    idx = np.asarray(inputs["indices_s"]).astype(np.int64)
    w1 = np.asarray(inputs["weight1"], dtype=np.float32)
    w2 = np.asarray(inputs["weight2"], dtype=np.float32)
    b1 = np.asarray(inputs["bias1"], dtype=np.float32)
    b2 = np.asarray(inputs["bias2"], dtype=np.float32)

    T = x.shape[0]
    E, IN, HID = w1.shape
    OUT = w2.shape[2]
    assert E == N_CORES
    bf = ml_dtypes.bfloat16

    order = np.argsort(idx, kind="stable")
    counts = np.bincount(idx, minlength=E)
    starts = np.zeros(E + 1, dtype=np.int64)
    starts[1:] = np.cumsum(counts)
    # Tokens are the matmul moving dim — no 128 alignment needed. Pad the
    # per-expert capacity to 16 only (DMA-friendly), not 128: with counts
    # ~1024+eps this cuts ~11% of padded compute.
    C = max(-(-int(counts.max()) // 16) * 16, 16)

    xbf = x.astype(bf)
    in_maps = []
    for e in range(E):
        toks = order[starts[e]:starts[e + 1]]
        xTe = np.zeros((IN, C), dtype=bf)
        if len(toks):
            xTe[:, :len(toks)] = xbf[toks].T
        in_maps.append({
            "xT": xTe,
            "w1": np.ascontiguousarray(w1[e]).astype(bf),
            "w2": np.ascontiguousarray(w2[e]).astype(bf),
            "b1": np.ascontiguousarray(b1[e]),
            "b2": np.ascontiguousarray(b2[e]),
        })
    meta = {"key": (C, IN, HID, OUT), "order": order, "starts": starts,
            "T": T, "OUT": OUT}
    return in_maps, meta


def scatter_output(inputs, yT_all, meta):
    """Scatter per-core yT [E, OUT, C] back to [T, 1, OUT] fp32."""
    order, starts = meta["order"], meta["starts"]
    out = np.empty((meta["T"], meta["OUT"]), dtype=np.float32)
    for e in range(N_CORES):
        toks = order[starts[e]:starts[e + 1]]
        if len(toks):
            out[toks] = yT_all[e][:, :len(toks)].T
    return out[:, None, :]


def kernel(**inputs):
    in_maps, meta = prepare_in_maps(inputs)
    nc = _get_kernel(*meta["key"])
    res = run_bass_kernel_spmd(nc, in_maps, core_ids=list(range(N_CORES)),
                               trace=False)
    yT_all = np.stack([res.results[e]["yT"] for e in range(N_CORES)])
    return scatter_output(inputs, yT_all, meta)



# revision 30
# speedup vs baseline: 1.4633x; 1.1624x over previous
"""MoE runtime-experts kernel for 8 Trainium2 NeuronCores.

Expert-parallel: core e holds expert e's weights. Host routes tokens by
expert id (argsort), pads each expert batch to a common capacity C, and
each core computes y = gelu(x @ W1 + b1) @ W2 + b2 for its batch as
dense matmuls in a transposed layout:

    L1: hT[hid, tok]  = W1[in, hid].T-contract  xT[in, tok]
    L2: yT[out, tok]  = W2[hid, out].T-contract hT[hid, tok]

Weights stay stationary on the PE (lhsT), tokens are the moving dim, so
activations flow through both layers without any on-device transpose.
Inputs/weights are cast to bf16 on host (PSUM accumulates fp32).

Key layout/scheduling choices (each measured against the cost-model
timeline and HW repeat-loop deltas):
  - C pads to 16, not 128: tokens are the moving dim and need no
    alignment; 128-padding burned 12.5% extra compute (max count 1040).
  - C splits into near-equal ~260-wide token tiles: maximizes the
    per-matmul moving width under the 512-fp32 PSUM bank limit and
    avoids a skinny tail tile whose 512 MMs pay the ~25ns issue floor.
    2048 matmuls total vs 4608 at 128-wide.
  - All of x resident in SBUF, DMA'd before the bulk weight windows:
    an x tile queued behind 16MB of weights stalled the PE ~18us.
  - w1 and w2 stream in m-major windows sized so arrival outpaces the
    PE's chain-by-chain demand; L1 runs two tiles ahead of L2 so the
    w2 stream stays covered even at ~30% below nominal HBM bandwidth.
  - A dozen dummy matmuls on zeroed scratch warm the PE's HAM clock
    gate (cold = 1.2GHz for ~3.4us) during the initial DMA wait.
"""

import numpy as np
import ml_dtypes

import concourse.bass as bass
import concourse.mybir as mybir
import concourse.tile as tile
from concourse import bacc
from concourse.bass_utils import run_bass_kernel_spmd

P = 128
N_CORES = 8
BF16 = mybir.dt.bfloat16
F32 = mybir.dt.float32

_nc_cache = {}


def _token_tile_widths(C, maxw=272):
    """Split C into near-equal chunks of <=maxw tokens. Tokens are the
    moving dim, so widths need no 128 alignment. Near-equal (vs 512+tail)
    keeps the per-MM NX overhead amortized evenly and avoids a skinny
    tail tile whose 512 matmuls would each pay the ~25ns issue floor.
    maxw=272 gives 4 near-equal tiles for C~1040; wider tiles save only
    ~1ns/MM of dispatch overhead (cost model) while eating the SBUF slack
    that lets L1 run two tiles ahead of L2."""
    nch = -(-C // maxw)
    base, rem = divmod(C, nch)
    widths = [base + (1 if i < rem else 0) for i in range(nch)]
    assert sum(widths) == C
    return widths


_MAXW = 272


def _build_kernel(C, IN, HID, OUT, skip_in_dma=False, psum_bufs=8, repeat=1,
                  tile_w=None, PIPE=2, GROUP=True, skip_out_dma=False):
    K1, M1 = IN // P, HID // P
    K2, M2 = HID // P, OUT // P
    if tile_w is not None:
        widths = [tile_w] * (C // tile_w)
        if C % tile_w:
            widths.append(C % tile_w)
    else:
        widths = _token_tile_widths(C, maxw=_MAXW)

    nc = bacc.Bacc("TRN2", target_bir_lowering=False, debug=False,
                   num_devices=N_CORES)
    xT = nc.dram_tensor("xT", [IN, C], BF16, kind="ExternalInput")
    w1 = nc.dram_tensor("w1", [IN, HID], BF16, kind="ExternalInput")
    w2 = nc.dram_tensor("w2", [HID, OUT], BF16, kind="ExternalInput")
    b1 = nc.dram_tensor("b1", [HID], F32, kind="ExternalInput")
    b2 = nc.dram_tensor("b2", [OUT], F32, kind="ExternalInput")
    yT = nc.dram_tensor("yT", [OUT, C], F32, kind="ExternalOutput")

    with tile.TileContext(nc) as tc:
        with (
            tc.tile_pool(name="weights", bufs=1) as wpool,
            tc.tile_pool(name="hbuf", bufs=PIPE + 1) as hpool,
            # obuf single-buffered: per-group y DMAs drain each o_sb region
            # ~28us before the next tile's bias-adds rewrite it, and the
            # freed 11KB/partition buys the third h buffer (PIPE=2).
            tc.tile_pool(name="obuf", bufs=1) as opool,
            tc.tile_pool(name="psum", bufs=psum_bufs, space="PSUM") as pspool,
        ):
            w1_sb = wpool.tile([P, K1, HID], BF16)
            w2_sb = wpool.tile([P, K2, OUT], BF16)
            b1_sb = wpool.tile([P, M1], F32)
            b2_sb = wpool.tile([P, M2], F32)
            # All of x stays resident (2.1MB): its DMA is issued before the
            # bulk weight windows so no L1 tile ever waits behind the 16MB
            # weight stream on the shared HBM bandwidth.
            x_sb = wpool.tile([P, K1, C], BF16)

            xTr = xT.ap().rearrange("(k p) c -> p k c", p=P)
            w1r = w1.ap().rearrange("(k p) m -> p k m", p=P)
            w2r = w2.ap().rearrange("(k p) m -> p k m", p=P)

            if repeat == 1:
                # HAM warm-up: the PE clock-gate sits at 4/8 (1.2GHz) until
                # ~3.4us of sustained activity. Burn that window on dummy
                # matmuls over a zeroed scratch tile while the first x/w1
                # DMAs are still in flight, so real work starts at 2.4GHz.
                warm_sb = wpool.tile([P, 260], BF16)
                nc.vector.memset(warm_sb[:], 0.0)
                wps = pspool.tile([P, 512], F32, tag="ps", name="ps")
                for _ in range(12):
                    nc.tensor.matmul(wps[:, :260], warm_sb[:, :P],
                                     warm_sb[:], start=True, stop=True,
                                     skip_group_check=True)

            def x_dma(head):
                # slice 0 first, on the Activation HWDGE queue — runs in
                # parallel with w1 window 0 on the SP queue.
                W0 = widths[0]
                nc.scalar.dma_start(x_sb[:, :, :W0], xTr[:, :, :W0])
                if not head:
                    nc.sync.dma_start(x_sb[:, :, widths[0]:],
                                      xTr[:, :, widths[0]:])

            if not skip_in_dma:
                if repeat == 1:
                    x_dma(head=True)
                # w1 chunked by m-window (all k per DMA): the first L1 chain
                # only needs window 0 instead of all of w1 (8MB), and
                # few big DMAs keep the per-DMA issue overhead
                # (~0.7us on the SP sequencer) off the critical path.
                # Small leading windows let the PE start ~3us earlier than
                # 512-col ones would.
                mw_edges = [0, 128, 256, 512]
                while mw_edges[-1] < HID:
                    mw_edges.append(min(HID, mw_edges[-1] + 512))
                nc.sync.dma_start(w1_sb[:, :, :128], w1r[:, :, :128])
                # Biases are tiny and the first gelu needs b1 early; they
                # ride the Activation queue alongside x.
                nc.scalar.dma_start(b1_sb[:],
                                    b1.ap().rearrange("(m p) -> p m", p=P))
                nc.scalar.dma_start(b2_sb[:],
                                    b2.ap().rearrange("(m p) -> p m", p=P))
                for lo, hi in zip(mw_edges[1:-1], mw_edges[2:]):
                    nc.sync.dma_start(w1_sb[:, :, lo:hi], w1r[:, :, lo:hi])
                # x remainder after w1: w1 windows feed L1 tile 0 chain-by-
                # chain (demand 0.9us/window vs supply 0.7us), while the x
                # remainder isn't read until L1 tile 1 (~34us in).
                if repeat == 1:
                    nc.sync.dma_start(x_sb[:, :, widths[0]:],
                                      xTr[:, :, widths[0]:])
                # w2 m-major windows: L2 chains for out-block mg only need
                # the window containing mg, so the first L2 chain can start
                # as soon as its window lands instead of waiting for all
                # 8MB of w2.
                MW = 512
                for mw in range(OUT // MW):
                    nc.sync.dma_start(
                        w2_sb[:, :, mw * MW:(mw + 1) * MW],
                        w2r[:, :, mw * MW:(mw + 1) * MW])

            starts_ = [sum(widths[:i]) for i in range(len(widths))]
            MAXW = max(widths)

            def l1_phase(it):
                NW, n0 = widths[it], starts_[it]
                h_sb = hpool.tile([P, K2, MAXW], BF16, tag="h", name="h_sb")
                # Pack as many accumulation chains as fit into one PSUM
                # bank (512 fp32) so the slot-WAR sem wait is amortized.
                spb = 4 if NW <= 128 else (2 if NW <= 256 else 1)
                slot = 512 // spb
                for mg in range(0, M1, spb):
                    ps = pspool.tile([P, 512], F32, tag="ps", name="ps")
                    for mi in range(spb):
                        m = mg + mi
                        for k in range(K1):
                            nc.tensor.matmul(
                                ps[:, mi * slot:mi * slot + NW],
                                w1_sb[:, k, bass.ts(m, P)],
                                x_sb[:, k, n0:n0 + NW],
                                start=(k == 0),
                                stop=(k == K1 - 1),
                            )
                    for mi in range(spb):
                        m = mg + mi
                        nc.scalar.activation(
                            h_sb[:, m, :NW],
                            ps[:, mi * slot:mi * slot + NW],
                            mybir.ActivationFunctionType.Gelu,
                            bias=b1_sb[:, m:m + 1],
                        )
                return h_sb

            yTr = yT.ap().rearrange("(m p) c -> p m c", p=P)

            def l2_phase(it, h_sb):
                NW, n0 = widths[it], starts_[it]
                o_sb = opool.tile([P, M2, MAXW], F32, tag="o", name="o_sb")
                spb = 4 if NW <= 128 else (2 if NW <= 256 else 1)
                slot = 512 // spb
                for mg in range(0, M2, spb):
                    ps = pspool.tile([P, 512], F32, tag="ps", name="ps")
                    for mi in range(spb):
                        m = mg + mi
                        for k in range(K2):
                            nc.tensor.matmul(
                                ps[:, mi * slot:mi * slot + NW],
                                w2_sb[:, k, bass.ts(m, P)],
                                h_sb[:, k, :NW],
                                start=(k == 0),
                                stop=(k == K2 - 1),
                            )
                    for mi in range(spb):
                        m = mg + mi
                        nc.vector.tensor_tensor(
                            o_sb[:, m, :NW],
                            ps[:, mi * slot:mi * slot + NW],
                            b2_sb[:, m:m + 1].to_broadcast((P, NW)),
                            mybir.AluOpType.add,
                        )
                    # Per-group output DMA overlaps the remaining chains'
                    # compute — shrinks the end-of-kernel drain.
                    if not skip_out_dma:
                        nc.sync.dma_start(yTr[:, mg:mg + spb, n0:n0 + NW],
                                          o_sb[:, mg:mg + spb, :NW])

            def body():
                # Software pipeline: L1 runs PIPE tiles ahead of L2 so the
                # w2 weight DMA tail hides behind L1 compute at startup.
                if repeat > 1 and not skip_in_dma:
                    x_dma(head=False)
                n_t = len(widths)
                depth = min(PIPE, n_t)
                hs = {}
                for it in range(depth):
                    hs[it] = l1_phase(it)
                for j in range(n_t):
                    if j + depth < n_t:
                        hs[j + depth] = l1_phase(j + depth)
                    l2_phase(j, hs.pop(j))

            if repeat == 1:
                body()
            else:
                with tc.For_i(0, repeat, 1, name="rep"):
                    body()
    nc.compile()
    return nc


def _get_kernel(C, IN, HID, OUT):
    key = (C, IN, HID, OUT)
    if key not in _nc_cache:
        _nc_cache[key] = _build_kernel(C, IN, HID, OUT)
    return _nc_cache[key]


def prepare_in_maps(inputs):
    """Host-side routing: sort tokens by expert, pad to capacity C,
    build per-core input maps. Returns (in_maps, meta)."""
    x = np.ascontiguousarray(np.asarray(inputs["x"], dtype=np.float32))
    idx = np.asarray(inputs["indices_s"]).astype(np.int64)
    w1 = np.asarray(inputs["weight1"], dtype=np.float32)
    w2 = np.asarray(inputs["weight2"], dtype=np.float32)
    b1 = np.asarray(inputs["bias1"], dtype=np.float32)
    b2 = np.asarray(inputs["bias2"], dtype=np.float32)

    T = x.shape[0]
    E, IN, HID = w1.shape
    OUT = w2.shape[2]
    assert E == N_CORES
    bf = ml_dtypes.bfloat16

    order = np.argsort(idx, kind="stable")
    counts = np.bincount(idx, minlength=E)
    starts = np.zeros(E + 1, dtype=np.int64)
    starts[1:] = np.cumsum(counts)
    # Tokens are the matmul moving dim — no 128 alignment needed. Pad the
    # per-expert capacity to 16 only (DMA-friendly), not 128: with counts
    # ~1024+eps this cuts ~11% of padded compute.
    C = max(-(-int(counts.max()) // 16) * 16, 16)

    xbf = x.astype(bf)
    in_maps = []
    for e in range(E):
        toks = order[starts[e]:starts[e + 1]]
        xTe = np.zeros((IN, C), dtype=bf)
        if len(toks):
            xTe[:, :len(toks)] = xbf[toks].T
        in_maps.append({
            "xT": xTe,
            "w1": np.ascontiguousarray(w1[e]).astype(bf),
            "w2": np.ascontiguousarray(w2[e]).astype(bf),
            "b1": np.ascontiguousarray(b1[e]),
            "b2": np.ascontiguousarray(b2[e]),
        })
    meta = {"key": (C, IN, HID, OUT), "order": order, "starts": starts,
            "T": T, "OUT": OUT}
    return in_maps, meta


def scatter_output(inputs, yT_all, meta):
    """Scatter per-core yT [E, OUT, C] back to [T, 1, OUT] fp32."""
    order, starts = meta["order"], meta["starts"]
    out = np.empty((meta["T"], meta["OUT"]), dtype=np.float32)
    for e in range(N_CORES):
        toks = order[starts[e]:starts[e + 1]]
        if len(toks):
            out[toks] = yT_all[e][:, :len(toks)].T
    return out[:, None, :]


def kernel(**inputs):
    in_maps, meta = prepare_in_maps(inputs)
    nc = _get_kernel(*meta["key"])
    res = run_bass_kernel_spmd(nc, in_maps, core_ids=list(range(N_CORES)),
                               trace=False)
    yT_all = np.stack([res.results[e]["yT"] for e in range(N_CORES)])
    return scatter_output(inputs, yT_all, meta)



# revision 31
# speedup vs baseline: 1.4669x; 1.0025x over previous
"""MoE runtime-experts kernel for 8 Trainium2 NeuronCores.

Expert-parallel: core e holds expert e's weights. Host routes tokens by
expert id (argsort), pads each expert batch to a common capacity C, and
each core computes y = gelu(x @ W1 + b1) @ W2 + b2 for its batch as
dense matmuls in a transposed layout:

    L1: hT[hid, tok]  = W1[in, hid].T-contract  xT[in, tok]
    L2: yT[out, tok]  = W2[hid, out].T-contract hT[hid, tok]

Weights stay stationary on the PE (lhsT), tokens are the moving dim, so
activations flow through both layers without any on-device transpose.
Inputs/weights are cast to bf16 on host (PSUM accumulates fp32).

Key layout/scheduling choices (each measured against the cost-model
timeline and HW repeat-loop deltas):
  - C pads to 16, not 128: tokens are the moving dim and need no
    alignment; 128-padding burned 12.5% extra compute (max count 1040).
  - C splits into near-equal ~260-wide token tiles: maximizes the
    per-matmul moving width under the 512-fp32 PSUM bank limit and
    avoids a skinny tail tile whose 512 MMs pay the ~25ns issue floor.
    2048 matmuls total vs 4608 at 128-wide.
  - All of x resident in SBUF, DMA'd before the bulk weight windows:
    an x tile queued behind 16MB of weights stalled the PE ~18us.
  - w1 and w2 stream in m-major windows sized so arrival outpaces the
    PE's chain-by-chain demand; L1 runs two tiles ahead of L2 so the
    w2 stream stays covered even at ~30% below nominal HBM bandwidth.
  - A dozen dummy matmuls on zeroed scratch warm the PE's HAM clock
    gate (cold = 1.2GHz for ~3.4us) during the initial DMA wait.
"""

import numpy as np
import ml_dtypes

import concourse.bass as bass
import concourse.mybir as mybir
import concourse.tile as tile
from concourse import bacc
from concourse.bass_utils import run_bass_kernel_spmd

P = 128
N_CORES = 8
BF16 = mybir.dt.bfloat16
F32 = mybir.dt.float32

_nc_cache = {}


def _token_tile_widths(C, maxw=272):
    """Split C into near-equal chunks of <=maxw tokens. Tokens are the
    moving dim, so widths need no 128 alignment. Near-equal (vs 512+tail)
    keeps the per-MM NX overhead amortized evenly and avoids a skinny
    tail tile whose 512 matmuls would each pay the ~25ns issue floor.
    maxw=272 gives 4 near-equal tiles for C~1040; wider tiles save only
    ~1ns/MM of dispatch overhead (cost model) while eating the SBUF slack
    that lets L1 run two tiles ahead of L2."""
    nch = -(-C // maxw)
    base, rem = divmod(C, nch)
    widths = [base + (1 if i < rem else 0) for i in range(nch)]
    assert sum(widths) == C
    return widths


_MAXW = 272


def _build_kernel(C, IN, HID, OUT, skip_in_dma=False, psum_bufs=8, repeat=1,
                  tile_w=None, PIPE=2, GROUP=True, skip_out_dma=False):
    K1, M1 = IN // P, HID // P
    K2, M2 = HID // P, OUT // P
    if tile_w is not None:
        widths = [tile_w] * (C // tile_w)
        if C % tile_w:
            widths.append(C % tile_w)
    else:
        widths = _token_tile_widths(C, maxw=_MAXW)

    nc = bacc.Bacc("TRN2", target_bir_lowering=False, debug=False,
                   num_devices=N_CORES)
    xT = nc.dram_tensor("xT", [IN, C], BF16, kind="ExternalInput")
    w1 = nc.dram_tensor("w1", [IN, HID], BF16, kind="ExternalInput")
    w2 = nc.dram_tensor("w2", [HID, OUT], BF16, kind="ExternalInput")
    b1 = nc.dram_tensor("b1", [HID], F32, kind="ExternalInput")
    b2 = nc.dram_tensor("b2", [OUT], F32, kind="ExternalInput")
    yT = nc.dram_tensor("yT", [OUT, C], F32, kind="ExternalOutput")

    with tile.TileContext(nc) as tc:
        with (
            tc.tile_pool(name="weights", bufs=1) as wpool,
            tc.tile_pool(name="hbuf", bufs=PIPE + 1) as hpool,
            # obuf single-buffered: per-group y DMAs drain each o_sb region
            # ~28us before the next tile's bias-adds rewrite it, and the
            # freed 11KB/partition buys the third h buffer (PIPE=2).
            tc.tile_pool(name="obuf", bufs=1) as opool,
            tc.tile_pool(name="psum", bufs=psum_bufs, space="PSUM") as pspool,
        ):
            w1_sb = wpool.tile([P, K1, HID], BF16)
            w2_sb = wpool.tile([P, K2, OUT], BF16)
            b1_sb = wpool.tile([P, M1], F32)
            b2_sb = wpool.tile([P, M2], F32)
            # All of x stays resident (2.1MB): its DMA is issued before the
            # bulk weight windows so no L1 tile ever waits behind the 16MB
            # weight stream on the shared HBM bandwidth.
            x_sb = wpool.tile([P, K1, C], BF16)

            xTr = xT.ap().rearrange("(k p) c -> p k c", p=P)
            w1r = w1.ap().rearrange("(k p) m -> p k m", p=P)
            w2r = w2.ap().rearrange("(k p) m -> p k m", p=P)

            if repeat == 1:
                # HAM warm-up: the PE clock-gate sits at 4/8 (1.2GHz) until
                # ~3.4us of sustained activity. Burn that window on dummy
                # matmuls over a zeroed scratch tile while the first x/w1
                # DMAs are still in flight, so real work starts at 2.4GHz.
                # 18 x 217ns (cold) spans the full 3.4us HAM SHORT window,
                # so the gate flips even if the first data DMA lands early.
                warm_sb = wpool.tile([P, 260], BF16)
                nc.vector.memset(warm_sb[:], 0.0)
                wps = pspool.tile([P, 512], F32, tag="ps", name="ps")
                for _ in range(18):
                    nc.tensor.matmul(wps[:, :260], warm_sb[:, :P],
                                     warm_sb[:], start=True, stop=True,
                                     skip_group_check=True)

            def x_dma(head):
                # slice 0 first, on the Activation HWDGE queue — runs in
                # parallel with w1 window 0 on the SP queue.
                W0 = widths[0]
                nc.scalar.dma_start(x_sb[:, :, :W0], xTr[:, :, :W0])
                if not head:
                    nc.sync.dma_start(x_sb[:, :, widths[0]:],
                                      xTr[:, :, widths[0]:])

            if not skip_in_dma:
                if repeat == 1:
                    x_dma(head=True)
                # w1 chunked by m-window (all k per DMA): the first L1 chain
                # only needs window 0 instead of all of w1 (8MB), and
                # few big DMAs keep the per-DMA issue overhead
                # (~0.7us on the SP sequencer) off the critical path.
                # Small leading windows let the PE start ~3us earlier than
                # 512-col ones would.
                mw_edges = [0, 128, 256, 512]
                while mw_edges[-1] < HID:
                    mw_edges.append(min(HID, mw_edges[-1] + 512))
                nc.sync.dma_start(w1_sb[:, :, :128], w1r[:, :, :128])
                # Biases are tiny and the first gelu needs b1 early; they
                # ride the Activation queue alongside x.
                nc.scalar.dma_start(b1_sb[:],
                                    b1.ap().rearrange("(m p) -> p m", p=P))
                nc.scalar.dma_start(b2_sb[:],
                                    b2.ap().rearrange("(m p) -> p m", p=P))
                for lo, hi in zip(mw_edges[1:-1], mw_edges[2:]):
                    nc.sync.dma_start(w1_sb[:, :, lo:hi], w1r[:, :, lo:hi])
                # x remainder after w1: w1 windows feed L1 tile 0 chain-by-
                # chain (demand 0.9us/window vs supply 0.7us), while the x
                # remainder isn't read until L1 tile 1 (~34us in).
                if repeat == 1:
                    nc.sync.dma_start(x_sb[:, :, widths[0]:],
                                      xTr[:, :, widths[0]:])
                # w2 m-major windows: L2 chains for out-block mg only need
                # the window containing mg, so the first L2 chain can start
                # as soon as its window lands instead of waiting for all
                # 8MB of w2.
                MW = 512
                for mw in range(OUT // MW):
                    nc.sync.dma_start(
                        w2_sb[:, :, mw * MW:(mw + 1) * MW],
                        w2r[:, :, mw * MW:(mw + 1) * MW])

            starts_ = [sum(widths[:i]) for i in range(len(widths))]
            MAXW = max(widths)

            def l1_phase(it):
                NW, n0 = widths[it], starts_[it]
                h_sb = hpool.tile([P, K2, MAXW], BF16, tag="h", name="h_sb")
                # Pack as many accumulation chains as fit into one PSUM
                # bank (512 fp32) so the slot-WAR sem wait is amortized.
                spb = 4 if NW <= 128 else (2 if NW <= 256 else 1)
                slot = 512 // spb
                for mg in range(0, M1, spb):
                    ps = pspool.tile([P, 512], F32, tag="ps", name="ps")
                    for mi in range(spb):
                        m = mg + mi
                        for k in range(K1):
                            nc.tensor.matmul(
                                ps[:, mi * slot:mi * slot + NW],
                                w1_sb[:, k, bass.ts(m, P)],
                                x_sb[:, k, n0:n0 + NW],
                                start=(k == 0),
                                stop=(k == K1 - 1),
                            )
                    for mi in range(spb):
                        m = mg + mi
                        nc.scalar.activation(
                            h_sb[:, m, :NW],
                            ps[:, mi * slot:mi * slot + NW],
                            mybir.ActivationFunctionType.Gelu,
                            bias=b1_sb[:, m:m + 1],
                        )
                return h_sb

            yTr = yT.ap().rearrange("(m p) c -> p m c", p=P)

            def l2_phase(it, h_sb):
                NW, n0 = widths[it], starts_[it]
                o_sb = opool.tile([P, M2, MAXW], F32, tag="o", name="o_sb")
                spb = 4 if NW <= 128 else (2 if NW <= 256 else 1)
                slot = 512 // spb
                for mg in range(0, M2, spb):
                    ps = pspool.tile([P, 512], F32, tag="ps", name="ps")
                    for mi in range(spb):
                        m = mg + mi
                        for k in range(K2):
                            nc.tensor.matmul(
                                ps[:, mi * slot:mi * slot + NW],
                                w2_sb[:, k, bass.ts(m, P)],
                                h_sb[:, k, :NW],
                                start=(k == 0),
                                stop=(k == K2 - 1),
                            )
                    for mi in range(spb):
                        m = mg + mi
                        nc.vector.tensor_tensor(
                            o_sb[:, m, :NW],
                            ps[:, mi * slot:mi * slot + NW],
                            b2_sb[:, m:m + 1].to_broadcast((P, NW)),
                            mybir.AluOpType.add,
                        )
                    # Per-group output DMA overlaps the remaining chains'
                    # compute — shrinks the end-of-kernel drain.
                    if not skip_out_dma:
                        nc.sync.dma_start(yTr[:, mg:mg + spb, n0:n0 + NW],
                                          o_sb[:, mg:mg + spb, :NW])

            def body():
                # Software pipeline: L1 runs PIPE tiles ahead of L2 so the
                # w2 weight DMA tail hides behind L1 compute at startup.
                if repeat > 1 and not skip_in_dma:
                    x_dma(head=False)
                n_t = len(widths)
                depth = min(PIPE, n_t)
                hs = {}
                for it in range(depth):
                    hs[it] = l1_phase(it)
                for j in range(n_t):
                    if j + depth < n_t:
                        hs[j + depth] = l1_phase(j + depth)
                    l2_phase(j, hs.pop(j))

            if repeat == 1:
                body()
            else:
                with tc.For_i(0, repeat, 1, name="rep"):
                    body()
    nc.compile()
    return nc


def _get_kernel(C, IN, HID, OUT):
    key = (C, IN, HID, OUT)
    if key not in _nc_cache:
        _nc_cache[key] = _build_kernel(C, IN, HID, OUT)
    return _nc_cache[key]


def prepare_in_maps(inputs):
    """Host-side routing: sort tokens by expert, pad to capacity C,
    build per-core input maps. Returns (in_maps, meta)."""
    x = np.ascontiguousarray(np.asarray(inputs["x"], dtype=np.float32))
    idx = np.asarray(inputs["indices_s"]).astype(np.int64)
    w1 = np.asarray(inputs["weight1"], dtype=np.float32)
    w2 = np.asarray(inputs["weight2"], dtype=np.float32)
    b1 = np.asarray(inputs["bias1"], dtype=np.float32)
    b2 = np.asarray(inputs["bias2"], dtype=np.float32)

    T = x.shape[0]
    E, IN, HID = w1.shape
    OUT = w2.shape[2]
    assert E == N_CORES
    bf = ml_dtypes.bfloat16

    order = np.argsort(idx, kind="stable")
    counts = np.bincount(idx, minlength=E)
    starts = np.zeros(E + 1, dtype=np.int64)
    starts[1:] = np.cumsum(counts)
    # Tokens are the matmul moving dim — no 128 alignment needed. Pad the
    # per-expert capacity to 16 only (DMA-friendly), not 128: with counts
    # ~1024+eps this cuts ~11% of padded compute.
    C = max(-(-int(counts.max()) // 16) * 16, 16)

    xbf = x.astype(bf)
    in_maps = []
    for e in range(E):
        toks = order[starts[e]:starts[e + 1]]
        xTe = np.zeros((IN, C), dtype=bf)
        if len(toks):
            xTe[:, :len(toks)] = xbf[toks].T
        in_maps.append({
            "xT": xTe,
            "w1": np.ascontiguousarray(w1[e]).astype(bf),
            "w2": np.ascontiguousarray(w2[e]).astype(bf),
            "b1": np.ascontiguousarray(b1[e]),
            "b2": np.ascontiguousarray(b2[e]),
        })
    meta = {"key": (C, IN, HID, OUT), "order": order, "starts": starts,
            "T": T, "OUT": OUT}
    return in_maps, meta


def scatter_output(inputs, yT_all, meta):
    """Scatter per-core yT [E, OUT, C] back to [T, 1, OUT] fp32."""
    order, starts = meta["order"], meta["starts"]
    out = np.empty((meta["T"], meta["OUT"]), dtype=np.float32)
    for e in range(N_CORES):
        toks = order[starts[e]:starts[e + 1]]
        if len(toks):
            out[toks] = yT_all[e][:, :len(toks)].T
    return out[:, None, :]


def kernel(**inputs):
    in_maps, meta = prepare_in_maps(inputs)
    nc = _get_kernel(*meta["key"])
    res = run_bass_kernel_spmd(nc, in_maps, core_ids=list(range(N_CORES)),
                               trace=False)
    yT_all = np.stack([res.results[e]["yT"] for e in range(N_CORES)])
    return scatter_output(inputs, yT_all, meta)

